# revision 53
# baseline (speedup 1.0000x reference)
"""Expert-parallel grouped GEMM (MoE) kernel for Trainium2.

Problem: out[e] = gelu(tok[e] @ w1[e]) @ w2[e]  per expert e.
  tok: [128, 2048, 128] f32, w1: [128, 128, 512] f32, w2: [128, 512, 128] f32.

Sharding: expert-parallel across 8 NeuronCores, 16 experts per core, no
cross-core communication. Each core runs the same Bass program on its own
expert slice (SPMD), the host concatenates the per-core outputs.

v2 dataflow (per core, per 512-token chunk):
  - tokens loaded via casting SWDGE DMA straight to bf16, natural [t, d]
    blocks (partition = t within a 128-token block)
  - PE-transpose token blocks to tokT [d, t] (bf16, 1 cyc/row), DVE copies
    PSUM -> SBUF (2x mode)
  - MM1 (bf16): hT[hd, t] = w1b.T @ tokT, into pair PSUM tiles [128, 2, 512]
  - GELU pair ops on ScalarE: PSUM f32 -> SBUF bf16 ht tiles
  - MM2 (bf16): po[t, o] += ht[hd-slice, t-block].T @ w2b[hd-slice]
    -- ht slices act as the (transposed-consumed) stationary, so the output
    lands in natural [t, o] layout: no output transposes at all
  - DVE drains po PSUM -> SBUF f32, SP HWDGE stores natural [t, o]
  - weights: f32 via SP HWDGE, DVE-cast to bf16 per expert
    (GPSIMD cannot touch PSUM, so Pool only issues the token SWDGE DMAs)

Every 4th chunk's last GELU tile is computed on DVE instead of ScalarE
(deg-2 polynomial with exact saturation, see POLY_*): the tile gets its
own 1-bank PSUM buffer (phd) so the DVE copy never blocks the ph
rotation, the polynomial spreads over 3 chunk slots behind the critical
tokt copy, and MM2 lags 3 chunks so it never waits on either GELU path.

Token assignment t = c*512 + 4p + j packs 4 consecutive tokens per
partition so the casting token DMA moves 2KB-read/1KB-write bursts at
full rate (the one-token-per-partition layout paid the <512B half-rate
penalty); the permutation rides through the whole pipe and is undone by
the store AP.

Steady-state (TimelineSim, 3x-unroll period): ~130.6 us/iter; Act 127.7,
PE 123.5, DVE 114.4, DMA 93.2. Measured HW slope: ~139-159 us/iter
depending on machine state (the v1 transpose-heavy baseline measured
198.5 in a comparable state).
"""

import numpy as np

NUM_CORES = 8
E_TOTAL = 128
E_PER_CORE = E_TOTAL // NUM_CORES  # 16
T = 2048
D = 128
H = 512
O = 128
P = 128

T_CHUNK = 512
N_CHUNKS = T // T_CHUNK  # 4
BLKS = T_CHUNK // P  # 4 token blocks per chunk
H_TILES = H // P  # 4

_CACHE = {}


DEFAULT_CFG = dict(
    tokb_bufs=3,
    tokc_bufs=4,
    tokt_bufs=3,
    ht_bufs=9,
    oc_bufs=4,
    w_bufs=2,
    pt_bufs=1,
    ph_bufs=2,
    po_bufs=2,
    pg_bufs=14,
    # divert the 4th GELU tile of every `dve_gelu`-th chunk to a DVE
    # polynomial (0 disables). Requires MM2 lag 3 so the poly parts,
    # spread over 3 chunk slots, never delay the tokt copy or MM2.
    dve_gelu=4,
    # additional diverts (g % pool_gelu == 3) with arithmetic on Pool.
    # Tried at 8: Pool's ~1.1us/op makes the slots lumpy enough to delay
    # its SWDGE token-DMA duties -> sim period 161us vs 130.6. Keep 0.
    pool_gelu=0,
)

# gelu(x) ~= x*(0.5 + clamp(x,+-B)*r(min(x^2, B^2))), r deg-2 Horner with
# r(B^2) = 0.5/B built in so the approximation saturates to x / 0 outside.
# Max abs err 0.019 on [-6, 6] (tolerance budget is ~0.104 abs).
POLY_C2 = 0.00147078
POLY_C1 = -0.03472488
POLY_C0 = 0.35761042547619215
POLY_B = 3.2
POLY_TB = POLY_B * POLY_B


def _build(loop=1, cfg=None):
    import concourse.bacc as bacc
    import concourse.mybir as mybir
    import concourse.tile as tile
    from concourse.masks import make_identity

    f32 = mybir.dt.float32
    bf16 = mybir.dt.bfloat16
    GELU = mybir.ActivationFunctionType.Gelu
    C = dict(DEFAULT_CFG)
    if cfg:
        C.update(cfg)

    nc = bacc.Bacc(
        "TRN2",
        target_bir_lowering=False,
        debug=False,
        num_devices=NUM_CORES,
    )

    tok = nc.dram_tensor(
        "group_token", [E_PER_CORE, T, D], f32, kind="ExternalInput"
    ).ap()
    w1 = nc.dram_tensor("weights1", [E_PER_CORE, D, H], f32, kind="ExternalInput").ap()
    w2 = nc.dram_tensor("weights2", [E_PER_CORE, H, O], f32, kind="ExternalInput").ap()
    out = nc.dram_tensor("out", [E_PER_CORE, T, O], f32, kind="ExternalOutput").ap()

    with tile.TileContext(nc) as tc:
        with (
            tc.tile_pool(name="const", bufs=1) as const_pool,
            tc.tile_pool(name="wf", bufs=C["w_bufs"]) as wf_pool,
            tc.tile_pool(name="wb", bufs=C["w_bufs"]) as wb_pool,
            tc.tile_pool(name="tokb", bufs=C["tokb_bufs"]) as tokb_pool,
            tc.tile_pool(name="tokc", bufs=C["tokc_bufs"]) as tokc_pool,
            tc.tile_pool(name="tokt", bufs=C["tokt_bufs"]) as tokt_pool,
            tc.tile_pool(name="ht", bufs=C["ht_bufs"]) as ht_pool,
            tc.tile_pool(name="pg", bufs=C["pg_bufs"]) as pg_pool,
            tc.tile_pool(name="oc", bufs=C["oc_bufs"]) as oc_pool,
            tc.tile_pool(name="pt", bufs=C["pt_bufs"], space="PSUM") as pt_pool,
            tc.tile_pool(name="ph", bufs=C["ph_bufs"], space="PSUM") as ph_pool,
            tc.tile_pool(name="po", bufs=C["po_bufs"], space="PSUM") as po_pool,
            tc.tile_pool(name="phd", bufs=1, space="PSUM") as phd_pool,
        ):
            ident_f32 = const_pool.tile([P, P], f32)
            make_identity(nc, ident_f32)
            ident = const_pool.tile([P, P], bf16)
            nc.vector.tensor_copy(ident[:], ident_f32[:])

            NG = E_PER_CORE * N_CHUNKS  # 64 global chunks

            def body(_iv=None):
                state = {}  # e -> (w1b, w2b)
                tokstate = {}  # e -> token tiles (set at setup_dma time)
                fstate = {}  # e -> (w1f, w2f) until the bf16 casts are emitted

                def setup_dma(e):
                    # tokens: casting DMA (gpsimd SWDGE) f32 -> bf16. Token
                    # assignment t = c*512 + 4p + j puts 4 consecutive tokens
                    # on each partition, so the DMA reads 2KB f32 / writes
                    # 1KB bf16 per burst (full rate; the old one-token-per-
                    # partition layout wrote 256B bursts at half rate). The
                    # permutation rides through transpose/MM1/GELU/MM2 as a
                    # consistent column order and is undone by the store AP.
                    # Expert 0's chunk-0 goes first: it gates the whole pipe.
                    if e == 0:
                        toks = []
                        for c in range(N_CHUNKS):
                            tkc = tokc_pool.tile(
                                [P, BLKS, D], bf16, tag="tokc", name=f"tokc{c}"
                            )
                            nc.gpsimd.dma_start(
                                tkc[:],
                                tok[e].rearrange(
                                    "(c p j) d -> c p j d", c=N_CHUNKS, p=P, j=BLKS
                                )[c],
                            )
                            toks.append(tkc)
                    else:
                        tf = tokb_pool.tile(
                            [P, N_CHUNKS, BLKS, D], bf16, tag="tokb", name=f"tokb{e}"
                        )
                        nc.gpsimd.dma_start(
                            tf[:],
                            tok[e].rearrange(
                                "(c p j) d -> p c j d", c=N_CHUNKS, p=P, j=BLKS
                            ),
                        )
                        toks = tf
                    # weights f32 via SP HWDGE; bf16 casts emitted later
                    # (setup_cast) so they don't delay the critical tokt copy
                    w1f = wf_pool.tile([P, H], f32, tag="w1f", name=f"w1f{e}")
                    nc.sync.dma_start(w1f[:], w1[e])
                    w2f = wf_pool.tile([P, H_TILES, O], f32, tag="w2f", name=f"w2f{e}")
                    nc.sync.dma_start(w2f[:], w2[e].rearrange("(k p) o -> p k o", p=P))
                    fstate[e] = (w1f, w2f)
                    tokstate[e] = toks

                def setup_cast(e):
                    # casts on Pool (SBUF->SBUF is allowed there) to keep DVE
                    # free for tokt/drains/poly-gelu
                    w1f, w2f = fstate.pop(e)
                    w1b = wb_pool.tile([P, H], bf16, tag="w1b", name=f"w1b{e}")
                    nc.gpsimd.tensor_copy(w1b[:], w1f[:])
                    w2b = wb_pool.tile([P, H_TILES, O], bf16, tag="w2b", name=f"w2b{e}")
                    nc.gpsimd.tensor_copy(w2b[:], w2f[:])
                    state[e] = (w1b, w2b)

                def blk(g, j):
                    e, c = divmod(g, N_CHUNKS)
                    toks = tokstate[e]
                    if isinstance(toks, list):
                        return toks[c][:, j]
                    return toks[:, c, j]

                pts = {}
                hts = {}
                pos = {}
                pending = {}  # slot g -> [poly part closures]
                Alu = mybir.AluOpType
                LAG = 3 if C.get("dve_gelu") or C.get("pool_gelu") else 1

                def dve_div(g):
                    n = C.get("dve_gelu")
                    return bool(n) and g % n == 1 and g + 2 < NG

                def pool_div(g):
                    # extra diverts whose SBUF arithmetic runs on Pool/GpSimd
                    # (slower per op but otherwise idle); DVE only does the
                    # PSUM x-copy, which GpSimd cannot
                    n = C.get("pool_gelu")
                    return bool(n) and g % n == 3 and g + 2 < NG and not dve_div(g)

                def diverted(g):
                    return dve_div(g) or pool_div(g)

                def schedule_poly(g, phd, ht, eng):
                    # gelu(phd) -> ht[:,1]. phd is a dedicated 1-bank PSUM
                    # tile so the x-copy (always DVE) never blocks the ph
                    # rotation; the arithmetic runs on `eng` spread over the
                    # next chunk slots to stay under that engine's period.
                    box = {}

                    def pg_tile(nm):
                        return pg_pool.tile(
                            [P, T_CHUNK], bf16, tag="pg", name=f"pg{nm}{g}"
                        )

                    box["x"] = x = pg_tile("x")
                    nc.vector.tensor_copy(x[:], phd[:])

                    def p_t():
                        box["t"] = t = pg_tile("t")
                        eng.tensor_mul(t[:], box["x"][:], box["x"][:])

                    def p_tbu():
                        box["tb"] = tb = pg_tile("tb")
                        eng.tensor_scalar_min(tb[:], box.pop("t")[:], POLY_TB)
                        box["u"] = u = pg_tile("u")
                        eng.tensor_scalar(
                            u[:], tb[:], POLY_C2, POLY_C1, Alu.mult, Alu.add
                        )

                    def p_rr2():
                        r = pg_tile("r")
                        eng.tensor_mul(r[:], box.pop("u")[:], box.pop("tb")[:])
                        box["r2"] = r2 = pg_tile("r2")
                        eng.tensor_scalar_add(r2[:], r[:], POLY_C0)

                    def p_s():
                        box["s"] = s = pg_tile("s")
                        eng.tensor_scalar(
                            s[:], box["x"][:], POLY_B, -POLY_B, Alu.min, Alu.max
                        )

                    def p_final():
                        x2 = box.pop("x")
                        w = pg_tile("w")
                        eng.tensor_mul(w[:], box.pop("s")[:], box.pop("r2")[:])
                        w2 = pg_tile("w2")
                        eng.tensor_scalar_add(w2[:], w[:], 0.5)
                        eng.tensor_mul(ht[:, 1], w2[:], x2[:])

                    if eng is nc.vector:
                        p_t()
                        pending.setdefault(g + 1, []).append(
                            lambda: (p_tbu(), p_rr2())
                        )
                        pending.setdefault(g + 2, []).append(
                            lambda: (p_s(), p_final())
                        )
                    else:
                        # Pool ops are slower (gpsimd efficiency): 3 per slot
                        pending.setdefault(g, []).append(lambda: (p_t(), p_tbu()))
                        pending.setdefault(g + 1, []).append(
                            lambda: (p_rr2(), p_s())
                        )
                        pending.setdefault(g + 2, []).append(p_final)

                def tin(g, j):
                    # PE transpose token block j of chunk g into pt[g]
                    if j == 0:
                        pts[g] = pt_pool.tile(
                            [P, T_CHUNK], bf16, tag="pt", name=f"pt{g}"
                        )
                    for _rep in range(2 if C.get("double_tin") else 1):
                        nc.tensor.transpose(
                            pts[g][:, j * P : (j + 1) * P], blk(g, j), ident[:]
                        )

                def mm2(g, b):
                    # po[:, b, :] += ht[hd].T @ w2b[hd] over 4 hd tiles
                    e = g // N_CHUNKS
                    w2b = state[e][1]
                    hta, htb = hts[g]
                    for _rep in range(2 if C.get("double_mm2") else 1):
                        for k in range(H_TILES):
                            src = hta if k < 2 else htb
                            nc.tensor.matmul(
                                pos[g][:, b],
                                src[:, k % 2, b * P : (b + 1) * P],
                                w2b[:, k],
                                start=(k == 0),
                                stop=(k == H_TILES - 1),
                            )

                def drain(g, half=None):
                    e, c = divmod(g, N_CHUNKS)
                    # t = c*512 + 4p + b: 4 tokens contiguous per partition
                    # (2KB stores), matching the load-side token assignment
                    dst = out[e].rearrange(
                        "(c p b) o -> c p b o", c=N_CHUNKS, p=P, b=BLKS
                    )[c]
                    if half is None:
                        oc = oc_pool.tile([P, BLKS, O], f32, tag="oc", name=f"oc{g}")
                        nc.vector.tensor_copy(oc[:], pos.pop(g)[:])
                        nc.sync.dma_start(dst, oc[:])
                        hts.pop(g)
                        return
                    hb = BLKS // 2
                    sl = slice(half * hb, (half + 1) * hb)
                    oc = oc_pool.tile(
                        [P, hb, O], f32, tag="oc", name=f"oc{g}_{half}"
                    )
                    nc.vector.tensor_copy(oc[:], pos[g][:, sl])
                    nc.sync.dma_start(dst[:, sl], oc[:])
                    if half == 1:
                        pos.pop(g)
                        hts.pop(g)

                setup_dma(0)
                setup_cast(0)
                for j in range(BLKS):
                    tin(0, j)

                hoisted = {}

                def get_dummy():
                    if "ht_dummy" not in state:
                        d = ht_pool.tile(
                            [P, 2, T_CHUNK], bf16, tag="htd", name="ht_dummy"
                        )
                        nc.vector.memset(d[:], 0.5)
                        state["ht_dummy"] = d
                    return state["ht_dummy"]

                def emit_pair(g, tokt, hp, pair_tiles):
                    # MM1 pair hp, its GELU (or poly divert), and after the
                    # first pair the next chunk's transposes so the tokt(g+1)
                    # chain starts as early as possible
                    w1b = state[g // N_CHUNKS][0]
                    ph = ph_pool.tile(
                        [P, 2, T_CHUNK], f32, tag="ph", name=f"ph{g}_{hp}"
                    )
                    dv = hp == 1 and diverted(g) and not C.get("skip_act")
                    phd = (
                        phd_pool.tile([P, T_CHUNK], f32, tag="phd", name=f"phd{g}")
                        if dv
                        else None
                    )
                    for _rep in range(2 if C.get("double_mm1") else 1):
                        for k in range(2):
                            hd = hp * 2 + k
                            dst = phd[:] if (dv and k == 1) else ph[:, k]
                            nc.tensor.matmul(
                                dst,
                                w1b[:, hd * P : (hd + 1) * P],
                                tokt[:],
                                start=True,
                                stop=True,
                            )
                    if C.get("skip_act"):
                        pair_tiles.append(get_dummy())
                    else:
                        ht = ht_pool.tile(
                            [P, 2, T_CHUNK], bf16, tag="ht", name=f"ht{g}_{hp}"
                        )
                        act = (
                            (lambda o, i: nc.scalar.copy(o, i))
                            if C.get("gelu_copy")
                            else (lambda o, i: nc.scalar.activation(o, i, GELU))
                        )
                        if dv:
                            # Act does only tile 0; poly covers tile 1
                            act(ht[:, 0], ph[:, 0])
                            schedule_poly(
                                g,
                                phd,
                                ht,
                                nc.vector if dve_div(g) else nc.gpsimd,
                            )
                        elif g == 0:
                            # startup: single-tile ops so Act starts sooner
                            act(ht[:, 0], ph[:, 0])
                            act(ht[:, 1], ph[:, 1])
                        else:
                            act(ht[:], ph[:])
                        if C.get("double_act"):
                            sink = ht_pool.tile(
                                [P, 2, T_CHUNK],
                                bf16,
                                tag="hts",
                                name=f"htsink{g}_{hp}",
                            )
                            act(sink[:], ph[:])
                        pair_tiles.append(ht)
                    if hp == 0 and g + 1 < NG:
                        for j in range(BLKS):
                            tin(g + 1, j)

                def chunk_front(g):
                    tokt = tokt_pool.tile(
                        [P, T_CHUNK], bf16, tag="tokt", name=f"tokt{g}"
                    )
                    nc.vector.tensor_copy(tokt[:], pts.pop(g)[:])
                    pair_tiles = []
                    emit_pair(g, tokt, 0, pair_tiles)
                    return tokt, pair_tiles

                for g in range(NG):
                    e, c = divmod(g, N_CHUNKS)
                    if c == 2 and e + 1 < E_PER_CORE:
                        setup_dma(e + 1)
                    if g in hoisted:
                        tokt, pair_tiles = hoisted.pop(g)
                    else:
                        tokt, pair_tiles = chunk_front(g)
                    emit_pair(g, tokt, 1, pair_tiles)
                    hts[g] = pair_tiles

                    # After a diverted chunk Act runs 426 ns ahead and would
                    # idle at the next MM1: hoist the next chunk's front half
                    # (tokt + MM1-a + GELU-a) ahead of this chunk's MM2 batch
                    # so Act's next dependency completes sooner. Diverts only
                    # occur at c==1, so g+1 is always the same expert.
                    if C.get("hoist") and diverted(g) and g + 1 < NG:
                        hoisted[g + 1] = chunk_front(g + 1)

                    # MM2 for the chunk LAG behind (its GELU — and poly,
                    # if diverted — finished at least one full chunk ago)
                    if g >= LAG:
                        gl = g - LAG
                        pos[gl] = po_pool.tile(
                            [P, BLKS, O], f32, tag="po", name=f"po{gl}"
                        )
                        for b in range(BLKS):
                            mm2(gl, b)
                        drain(gl)
                    # this slot's share of any in-flight DVE poly-gelu
                    for part in pending.pop(g, ()):
                        part()
                    if c == 3 and e + 1 < E_PER_CORE:
                        setup_cast(e + 1)

                # tail: remaining LAG chunks; last one drained in halves
                for gg in range(NG - LAG, NG):
                    pos[gg] = po_pool.tile([P, BLKS, O], f32, tag="po", name=f"po{gg}")
                    if gg == NG - 1:
                        mm2(gg, 0)
                        mm2(gg, 1)
                        drain(gg, half=0)
                        mm2(gg, 2)
                        mm2(gg, 3)
                        drain(gg, half=1)
                    else:
                        for b in range(BLKS):
                            mm2(gg, b)
                        drain(gg)

            if C.get("unroll"):
                for _ in range(int(C["unroll"])):
                    body()
            elif loop == 1:
                body()
            else:
                with tc.For_i(0, loop, 1) as _i:
                    body(_i)

    nc.compile()
    return nc


def _get_nc(loop=1, cfg=None):
    key = ("nc", loop, tuple(sorted((cfg or {}).items())))
    if key not in _CACHE:
        _CACHE[key] = _build(loop, cfg)
    return _CACHE[key]


def kernel(group_token, weights1, weights2):
    from concourse.bass_utils import run_bass_kernel_spmd

    group_token = np.ascontiguousarray(np.asarray(group_token, dtype=np.float32))
    weights1 = np.ascontiguousarray(np.asarray(weights1, dtype=np.float32))
    weights2 = np.ascontiguousarray(np.asarray(weights2, dtype=np.float32))

    nc = _get_nc()
    in_maps = []
    for c in range(NUM_CORES):
        sl = slice(c * E_PER_CORE, (c + 1) * E_PER_CORE)
        in_maps.append(
            {
                "group_token": np.ascontiguousarray(group_token[sl]),
                "weights1": np.ascontiguousarray(weights1[sl]),
                "weights2": np.ascontiguousarray(weights2[sl]),
            }
        )

    res = run_bass_kernel_spmd(nc, in_maps, core_ids=list(range(NUM_CORES)))
    _CACHE["last_results"] = res
    return np.concatenate([r["out"] for r in res.results], axis=0)
